# revision 26
# baseline (speedup 1.0000x reference)
"""AttentionLSTM Trainium2 kernel — 8-core data-parallel.

Model (per batch row b): two independent single-direction LSTMs over T=43
steps of x[:, :, t] (H=300 features), hidden states summed, then a
conv-softmax attention over time, tanh, fc(300->80), softmax.

Device mapping per core (512 batch rows):
  - z^T[1200, 512] per (direction, step) via PE matmuls: merged contraction
    K=5 k-tiles of 128 (h rows 0..299 first, then bias + x tail, then x),
    M gate-aligned tiles {128,128,44}, fp16 MM inputs, fp32 PSUM accum.
  - h k-tiles are parity double-buffered: step t reads kt[t%2], writes h_t
    into kt[(t+1)%2], so every matmul of a step sees the full h_{t-1}
    (exact LSTM semantics, no Gauss-Seidel staleness).
  - group order is d-major (all of direction 0, then direction 1): d0's
    elementwise tail hides under d1's matmuls and vice versa across the
    step boundary, keeping the PE dense.
  - gates: one fused sigmoid over an [*,1024] i|f PSUM pair, tanh(g) and
    sigmoid(o) over a shared g|o PSUM pair; gate/cell elementwise state in
    fp16 for 2x DVE throughput; c stays in SBUF.
  - attention accumulated online: e_t = sigmoid(a)/(1-sigmoid(a)) = exp(a)
    (avoids exp table loads mid-loop); e_t broadcast across partitions with
    a rank-1 PE matmul (ones x e) into PSUM — no DRAM round trip; r += on
    GPSIMD.
  - tail: hStar = tanh(r/s), logits = fc(hStar) via PE (batch on PSUM
    partitions), softmax over the 80-class free dim.
"""

import os
import sys

sys.path.insert(0, "/opt/trn_rl_repo")

from contextlib import ExitStack

import numpy as np

import concourse.bass as bass
import concourse.tile as tile
from concourse import mybir
from concourse.bass_utils import run_bass_kernel_spmd  # noqa: F401  (spmd path kept available)

f32 = mybir.dt.float32
AF = mybir.ActivationFunctionType
AX = mybir.AxisListType

_BIRFIX_DONE = False


def _split_multiwaits(bir_json):
    """This walrus build allows one sync-wait per engine instruction; Tile
    attaches one per producer proc. Hoist extras onto standalone
    EventSemaphore instructions inserted just before, same engine queue."""
    import json
    j = json.loads(bir_json.decode() if isinstance(bir_json, bytes) else bir_json)
    for fn in j.get("functions", []):
        for blk in fn.get("blocks", []):
            out = []
            for ins in blk.get("instructions", []):
                si = ins.get("sync_info")
                ow = si.get("on_wait") if si else None
                if ow and len(ow) > 1:
                    for i, w in enumerate(ow[:-1]):
                        out.append({
                            "debug": ins.get("debug", 0),
                            "engine": ins["engine"],
                            "ins": [], "outs": [],
                            "name": f"{ins['name']}_xw{i}",
                            "opcode": "EventSemaphore",
                            "sync_info": {"on_update": [], "on_wait": [w]},
                        })
                    si["on_wait"] = [ow[-1]]
                out.append(ins)
            blk["instructions"] = out
    return json.dumps(j).encode()


def _install_birfix():
    global _BIRFIX_DONE
    if _BIRFIX_DONE:
        return
    from concourse import bass2jax
    orig = bass2jax.compile_bir_kernel

    def patched(bir_json, tmpdir, neff_name="file.neff"):
        return orig(_split_multiwaits(bir_json), tmpdir, neff_name)

    bass2jax.compile_bir_kernel = patched
    _BIRFIX_DONE = True


class _Runner:
    """Compile once; keep the sharded jitted executable + device inputs."""

    def __init__(self, nc, n_cores):
        import jax
        from jax.sharding import Mesh, PartitionSpec
        from jax.experimental.shard_map import shard_map
        from concourse import bass2jax as b2j

        b2j.install_neuronx_cc_hook()
        _install_birfix()
        self.jax = jax
        self.nc = nc
        self.n_cores = n_cores
        part_name = nc.partition_id_tensor.name if nc.partition_id_tensor else None
        in_names, out_names, out_avals, zero_outs = [], [], [], []
        for alloc in nc.m.functions[0].allocations:
            if not isinstance(alloc, mybir.MemoryLocationSet):
                continue
            name = alloc.memorylocations[0].name
            if alloc.kind == "ExternalInput":
                if name != part_name:
                    in_names.append(name)
            elif alloc.kind == "ExternalOutput":
                out_names.append(name)
                shape = tuple(alloc.tensor_shape)
                dtype = mybir.dt.np(alloc.dtype)
                out_avals.append(jax.core.ShapedArray(shape, dtype))
                zero_outs.append(np.zeros(shape, dtype))
        self.in_names = list(in_names)
        self.out_names = out_names
        self.out_avals = out_avals
        self.zero_outs = zero_outs
        n_params = len(in_names)
        n_outs = len(out_avals)
        all_names = in_names + out_names
        if part_name is not None:
            all_names = all_names + [part_name]
        donate = tuple(range(n_params, n_params + n_outs))

        def _body(*args):
            operands = list(args)
            if part_name is not None:
                operands.append(b2j.partition_id_tensor())
            outs = b2j._bass_exec_p.bind(
                *operands,
                out_avals=tuple(out_avals),
                in_names=tuple(all_names),
                out_names=tuple(out_names),
                lowering_input_output_aliases=(),
                sim_require_finite=True,
                sim_require_nnan=True,
                nc=nc,
            )
            return tuple(outs)

        devices = jax.devices()[:n_cores]
        self.mesh = Mesh(np.asarray(devices), ("core",))
        in_specs = (PartitionSpec("core"),) * (n_params + n_outs)
        out_specs = (PartitionSpec("core"),) * n_outs
        self.sharded = jax.jit(
            shard_map(_body, mesh=self.mesh, in_specs=in_specs,
                      out_specs=out_specs, check_rep=False),
            donate_argnums=donate, keep_unused=True)
        self.sharding = jax.sharding.NamedSharding(
            self.mesh, PartitionSpec("core"))

    def put_inputs(self, in_maps):
        jax = self.jax
        concat = [np.concatenate([np.asarray(m[n]) for m in in_maps], axis=0)
                  for n in self.in_names]
        return [jax.device_put(a, self.sharding) for a in concat]

    def call(self, dev_in):
        zeros = [np.zeros((self.n_cores * z.shape[0], *z.shape[1:]), z.dtype)
                 for z in self.zero_outs]
        outs = self.sharded(*dev_in, *zeros)
        self.jax.block_until_ready(outs)
        return outs

    def run(self, in_maps):
        dev_in = self.put_inputs(in_maps)
        outs = self.call(dev_in)
        n = self.n_cores
        return [
            {name: np.asarray(outs[i]).reshape(n, *self.out_avals[i].shape)[c]
             for i, name in enumerate(self.out_names)}
            for c in range(n)
        ]

    def bench(self, in_maps, iters=5):
        import time
        dev_in = self.put_inputs(in_maps)
        self.call(dev_in)  # warm
        times = []
        for _ in range(iters):
            t0 = time.perf_counter()
            self.call(dev_in)
            times.append(time.perf_counter() - t0)
        return times


B, H, T, NCLS = 4096, 300, 43, 80
NCORES = 8
BS = B // NCORES          # 512 batch rows per core
NK = 5                    # k-tiles: [h0:128 | h128:256 | h256:300+bias+xtail | x0:128 | x128:256]
BIASROW = 44              # partition of the bias (constant-1) row in k-tile 2
XTAIL = 64                # x rows 256..300 live at parts 64..108 of k-tile 2
MT = [(0, 128), (128, 128), (256, 44)]    # (moff, msz) per gate, output base partition 0
GOFF = [0, 300, 600, 900]                 # torch gate order i,f,g,o
WDRW = 4 * 304                            # fp8 DR weights: 16B-aligned gate pitch

MM_DT_NAME = os.environ.get("LSTM_MM_DT", "float16")
# fp8e4m3 DoubleRow for the x[0:256] contraction: one 256-row matmul pass
# replaces two fp16 128-row passes (20% fewer gate matmul passes; emulated
# end-to-end rel err 8.2e-3 vs the 2e-2 gate).
USE_DR = os.environ.get("LSTM_X_FP8_DR", "1") == "1"

_CACHE = {}


def _build(mdt_name, repeat=0, variant="full"):
    # variant: "full" | "no_attn" (skip attention accumulation) |
    # "no_dve" (also skip the c/h elementwise chain) | "mm_only"
    # (matmuls + DMAs only) | "mm_nodma" (matmuls, static rhs) |
    # "mm_n256" (matmuls at N=256).  Non-"full" variants are timing probes.
    do_attn = variant == "full"
    do_dve = variant in ("full", "no_attn", "no_rec")
    do_act = variant not in ("mm_only", "mm_nodma", "mm_n256")
    do_xdma = variant != "mm_nodma"
    do_rec = variant != "no_rec"
    ncols = 256 if variant == "mm_n256" else 512
    mdt = getattr(mybir.dt, mdt_name)
    f8 = mybir.dt.float8e4
    DRMODE = mybir.MatmulPerfMode.DoubleRow
    nc = bass.Bass(target_bir_lowering=False)

    xt_d = nc.declare_dram_parameter("xt", [T, 3, 128, BS], mdt, isOutput=False)
    if USE_DR:
        xdr_d = nc.declare_dram_parameter("xdr", [T, 128, 2, BS], f8,
                                          isOutput=False)
        wdr_d = nc.declare_dram_parameter("wdr", [2, 128, 2, WDRW], f8,
                                          isOutput=False)
    wc_d = nc.declare_dram_parameter("wc", [2, NK, 128, 1200], mdt, isOutput=False)
    conv_d = nc.declare_dram_parameter("convp", [128, 3], mdt, isOutput=False)
    fcw_d = nc.declare_dram_parameter("fcw", [128, 3 * NCLS], mdt, isOutput=False)
    fcb_d = nc.declare_dram_parameter("fcb", [1, NCLS], mdt, isOutput=False)
    ones_d = nc.declare_dram_parameter("onesrow", [1, BS], mdt, isOutput=False)
    out_d = nc.declare_dram_parameter("out", [BS, NCLS], f32, isOutput=True)

    with tile.TileContext(nc) as tc, ExitStack() as ctx:
        P = lambda name, bufs, **kw: ctx.enter_context(
            tc.tile_pool(name=name, bufs=bufs, **kw))
        wpool = P("w", 1)
        xpool = P("x", 3)
        # One shared pool for all gate PSUM tiles: 3 x [128,1024] f32 =
        # 6 banks.  With separate zif(bufs=2)/zgo(bufs=1) pools the g|o
        # matmuls of each group waited on the previous group's o-act drain
        # with ~0 margin -> ~1us PE stall per group (~260us/forward).
        zp = P("z", 3, space="PSUM")
        # Attention score [1,512] and broadcast [128,512] share one slot
        # tag (strictly sequential within a step); 2 bufs = 2 banks.
        atp = P("at", 2, space="PSUM")
        sifp = P("sif", 3)
        sop = P("so", 3)
        gcp = P("gc", 1)
        p1p = P("p1", 3)
        tcp = P("tc", 3)
        hp = P("h", 1)
        hsp = P("hs", 2)
        thp = P("th", 2)
        rp = P("r", 1)
        smp = P("sm", 2)
        tmpp = P("tmp", 2)
        fin = P("fin", 2)

        # ---- weights / constants ----
        wc_sb = {}
        nk_sb = 3 if USE_DR else NK
        for d in range(2):
            for k in range(nk_sb):
                wt = wpool.tile([128, 1200], mdt, tag=f"wc_{d}_{k}")
                nc.sync.dma_start(out=wt, in_=wc_d.ap()[d, k])
                wc_sb[(d, k)] = wt
        wdr_sb = {}
        if USE_DR:
            for d in range(2):
                wt = wpool.tile([128, 2, WDRW], f8, tag=f"wdr_{d}")
                nc.sync.dma_start(out=wt, in_=wdr_d.ap()[d])
                wdr_sb[d] = wt
        conv_sb = wpool.tile([128, 3], mdt, tag="conv")
        nc.sync.dma_start(out=conv_sb, in_=conv_d.ap())
        fcw_sb = wpool.tile([128, 3 * NCLS], mdt, tag="fcw")
        nc.sync.dma_start(out=fcw_sb, in_=fcw_d.ap())
        fcb_sb = wpool.tile([1, NCLS], mdt, tag="fcb")
        nc.sync.dma_start(out=fcb_sb, in_=fcb_d.ap())
        ones_sb = wpool.tile([1, 128], mdt, tag="ones")
        nc.vector.memset(ones_sb, 1.0)

        # ---- persistent state ----
        # h k-tiles, parity double-buffered: step t reads kt[t%2][d],
        # writes h_t into kt[(t+1)%2][d].
        kt = {}
        for par in range(2):
            for d in range(2):
                kt[(par, d)] = []
                for j in range(3):
                    t_ = hp.tile([128, BS], mdt, tag=f"kt_{par}_{d}_{j}")
                    nc.vector.memset(t_, 0.0)
                    kt[(par, d)].append(t_)
                nc.sync.dma_start(out=kt[(par, d)][2][BIASROW:BIASROW + 1],
                                  in_=ones_d.ap())
        gc = {}    # gc[(d, j)]: [128, 1024] mdt = [tanh_g | c]
        for d in range(2):
            for j in range(3):
                g = gcp.tile([128, 1024], mdt, tag=f"gc_{d}_{j}")
                nc.vector.memset(g, 0.0)
                gc[(d, j)] = g
        r = []
        for j in range(3):
            rt = rp.tile([128, BS], f32, tag=f"r_{j}")
            nc.vector.memset(rt, 0.0)
            r.append(rt)
        ssum = rp.tile([1, BS], f32, tag="ssum")
        # timing variants skip attention: keep 1/ssum finite in the tail
        nc.vector.memset(ssum, 0.0 if do_attn else 1.0)

        def w_slice(d, k, col0, msz):
            return wc_sb[(d, k)][:, col0:col0 + msz]

        def attn_tanh(hs):
            # hs[j]: [128, BS] mdt hsum tiles from the PREVIOUS step.
            th = []
            for j in range(3):
                pmax = 45 if j == 2 else 128
                thj = thp.tile([128, BS], mdt, tag=f"th{j}")
                nc.scalar.activation(out=thj[0:pmax], in_=hs[j][0:pmax],
                                     func=AF.Tanh)
                th.append((thj, pmax))
            return th

        def attn_score(th):
            a_ps = atp.tile([1, BS], f32, tag="at")
            for k in range(3):
                thj, pmax = th[k]
                nc.tensor.matmul(a_ps, lhsT=conv_sb[0:pmax, k:k + 1],
                                 rhs=thj[0:pmax], start=(k == 0), stop=(k == 2))
            sg = smp.tile([1, BS], f32, tag="sg")
            nc.scalar.activation(out=sg, in_=a_ps, func=AF.Sigmoid)
            om = smp.tile([1, BS], f32, tag="om")
            nc.scalar.activation(out=om, in_=sg, func=AF.Copy, bias=1.0,
                                 scale=-1.0)
            nc.vector.reciprocal(out=om, in_=om)
            e = smp.tile([1, BS], f32, tag="e")
            nc.vector.tensor_mul(out=e, in0=sg, in1=om)   # e = exp(a)
            nc.vector.tensor_add(out=ssum, in0=ssum, in1=e)
            e16 = smp.tile([1, BS], mdt, tag="e16")
            nc.scalar.activation(out=e16, in_=e, func=AF.Copy)
            return e16

        def attn_accum(hs, e16):
            eb_ps = atp.tile([128, BS], f32, tag="at")
            nc.tensor.matmul(eb_ps, lhsT=ones_sb, rhs=e16, start=True, stop=True)
            for j in range(3):
                pmax = 45 if j == 2 else 128
                tmp = tmpp.tile([128, BS], f32, tag=f"tmp{j}")
                nc.vector.tensor_mul(out=tmp[0:pmax], in0=hs[j][0:pmax],
                                     in1=eb_ps[0:pmax])
                nc.gpsimd.tensor_add(out=r[j][0:pmax], in0=r[j][0:pmax],
                                     in1=tmp[0:pmax])

        def attn_tail(hs):
            attn_accum(hs, attn_score(attn_tanh(hs)))

        loop_cm = tc.For_i(0, repeat, 1) if repeat else None
        if loop_cm is not None:
            loop_cm.__enter__()

        pending_hs = None
        if not do_xdma:
            if USE_DR:
                xdr0 = xpool.tile([128, 2, BS], f8, tag="xdr")
                nc.vector.memset(xdr0, 0.0)
            else:
                xa0 = xpool.tile([128, BS], mdt, tag="xa")
                nc.vector.memset(xa0, 0.0)
                xb0 = xpool.tile([128, BS], mdt, tag="xb")
                nc.vector.memset(xb0, 0.0)

        # ---- time loop ----
        for t in range(T):
            par, nxt = t % 2, (t + 1) % 2
            xa = xb = xdr = None
            if do_xdma:
                if USE_DR:
                    xdr = xpool.tile([128, 2, BS], f8, tag="xdr")
                    nc.sync.dma_start(out=xdr, in_=xdr_d.ap()[t])
                else:
                    xa = xpool.tile([128, BS], mdt, tag="xa")
                    nc.sync.dma_start(out=xa, in_=xt_d.ap()[t, 0])
                    xb = xpool.tile([128, BS], mdt, tag="xb")
                    nc.sync.dma_start(out=xb, in_=xt_d.ap()[t, 1])
                for d in range(2):
                    nc.sync.dma_start(out=kt[(par, d)][2][XTAIL:XTAIL + 44],
                                      in_=xt_d.ap()[t, 2][XTAIL:XTAIL + 44])
            elif USE_DR:
                xdr = xdr0
            else:
                xa, xb = xa0, xb0
            # previous step's attention tanh: emitted first so the th acts
            # drain ahead of this step's gate acts in the ACT FIFO.
            pend_th = attn_tanh(pending_hs) if (do_attn and pending_hs) else None
            pend_e16 = None

            hs = []
            for d in range(2):
                rhsk = [kt[(par, d)][0], kt[(par, d)][1], kt[(par, d)][2]]
                if not USE_DR:
                    rhsk += [xa, xb]
                for j, (moff, msz) in enumerate(MT):
                    sl = slice(0, msz)
                    zif = zp.tile([128, 1024], f32, tag="z")
                    zgo = zp.tile([128, 1024], f32, tag="z")
                    for gi, zdst in ((0, zif[sl, 0:ncols]),
                                     (1, zif[sl, 512:512 + ncols]),
                                     (2, zgo[sl, 0:ncols]),
                                     (3, zgo[sl, 512:512 + ncols])):
                        col0 = GOFF[gi] + moff
                        if USE_DR:
                            # x[0:256] fp8 DoubleRow pass FIRST: the group's
                            # opening pass depends only on the x DMA, never
                            # on the previous step's DVE h-writes; h k-tiles
                            # follow in production order (j2-mix last).
                            dc0 = 304 * gi + moff   # 16B-aligned gate starts
                            nc.tensor.matmul(
                                zdst, lhsT=wdr_sb[d][:, :, dc0:dc0 + msz],
                                rhs=xdr[:, :, 0:ncols],
                                start=True, stop=False, perf_mode=DRMODE)
                        for k in range(len(rhsk)):
                            nc.tensor.matmul(
                                zdst, lhsT=w_slice(d, k, col0, msz),
                                rhs=rhsk[k][:, 0:ncols],
                                start=(not USE_DR and k == 0),
                                stop=(k == len(rhsk) - 1))
                    if not do_act:
                        continue
                    sif = sifp.tile([128, 1024], mdt, tag="sif")
                    nc.scalar.activation(out=sif[sl], in_=zif[sl],
                                         func=AF.Sigmoid)
                    gcj = gc[(d, j)]
                    nc.scalar.activation(out=gcj[sl, 0:512], in_=zgo[sl, 0:512],
                                         func=AF.Tanh)
                    so = sop.tile([128, BS], mdt, tag="so")
                    nc.scalar.activation(out=so[sl], in_=zgo[sl, 512:1024],
                                         func=AF.Sigmoid)
                    if not do_dve:
                        continue
                    # c_new = sig_f * c + sig_i * tanh_g ; h = sig_o * tanh(c)
                    p1 = p1p.tile([128, 1024], mdt, tag="p1")
                    nc.vector.tensor_mul(out=p1[sl], in0=sif[sl], in1=gcj[sl])
                    nc.vector.tensor_add(out=gcj[sl, 512:1024],
                                         in0=p1[sl, 0:512], in1=p1[sl, 512:1024])
                    tcj = tcp.tile([128, BS], mdt, tag="tc")
                    nc.scalar.activation(out=tcj[sl], in_=gcj[sl, 512:1024],
                                         func=AF.Tanh)
                    # h_t lands directly in the next step's rhs k-tile
                    if do_rec:
                        hdst = kt[(nxt, d)][j]
                    else:  # timing probe: same traffic, no recurrence dep
                        hdst = tcp.tile([128, BS], mdt, tag="hscr")
                    nc.vector.tensor_mul(out=hdst[sl], in0=so[sl],
                                         in1=tcj[sl])
                    if d == 1 and do_attn:
                        pmax = 45 if j == 2 else 128
                        hsj = hsp.tile([128, BS], mdt, tag=f"hs{j}")
                        nc.vector.tensor_add(out=hsj[0:pmax],
                                             in0=kt[(nxt, 0)][j][0:pmax],
                                             in1=kt[(nxt, 1)][j][0:pmax])
                        hs.append(hsj)
                    # previous step's attention, staged so its PE ops never
                    # wait on its ACT/DVE chain: score after d0-j0 (conv
                    # matmuls see ready th, e16 chain overlaps d0-j1/j2),
                    # accumulate after d1-j0 (eb matmul sees ready e16).
                    if pend_th is not None:
                        if d == 0 and j == 1:
                            pend_e16 = attn_score(pend_th)
                        elif d == 1 and j == 0:
                            attn_accum(pending_hs, pend_e16)
            pending_hs = hs

        if do_attn:
            attn_tail(pending_hs)

        if loop_cm is not None:
            loop_cm.__exit__(None, None, None)

        # ---- tail: hStar = tanh(r / s); logits; softmax ----
        rs = smp.tile([1, BS], f32, tag="rs")
        nc.vector.reciprocal(out=rs, in_=ssum)
        rs16 = smp.tile([1, BS], mdt, tag="rs16")
        nc.scalar.activation(out=rs16, in_=rs, func=AF.Copy)
        rsb = atp.tile([128, BS], f32, tag="at")
        nc.tensor.matmul(rsb, lhsT=ones_sb, rhs=rs16, start=True, stop=True)
        hst = []
        for j in range(3):
            hn = fin.tile([128, BS], f32, tag=f"hn{j}")
            nc.vector.tensor_mul(out=hn, in0=r[j], in1=rsb)
            hj = fin.tile([128, BS], mdt, tag=f"hst{j}")
            nc.scalar.activation(out=hj, in_=hn, func=AF.Tanh)
            hst.append(hj)
        for bt in range(BS // 128):
            fcp = atp.tile([128, NCLS], f32, tag="at")
            for j in range(3):
                nc.tensor.matmul(fcp, lhsT=hst[j][:, bt * 128:(bt + 1) * 128],
                                 rhs=fcw_sb[:, j * NCLS:(j + 1) * NCLS],
                                 start=(j == 0), stop=False)
            nc.tensor.matmul(fcp, lhsT=ones_sb, rhs=fcb_sb, start=False, stop=True)
            mx = fin.tile([128, 1], f32, tag="mx")
            nc.vector.reduce_max(out=mx, in_=fcp, axis=AX.X)
            nmx = fin.tile([128, 1], f32, tag="nmx")
            nc.vector.tensor_scalar_mul(out=nmx, in0=mx, scalar1=-1.0)
            ex = fin.tile([128, NCLS], f32, tag="ex")
            nc.scalar.activation(out=ex, in_=fcp, func=AF.Exp, bias=nmx)
            sm = fin.tile([128, 1], f32, tag="smm")
            nc.vector.reduce_sum(out=sm, in_=ex, axis=AX.X)
            nc.vector.reciprocal(out=sm, in_=sm)
            ot = fin.tile([128, NCLS], f32, tag="ot")
            nc.vector.tensor_scalar_mul(out=ot, in0=ex, scalar1=sm)
            nc.sync.dma_start(out=out_d.ap()[bt * 128:(bt + 1) * 128], in_=ot)

    return nc


def _prep(x, w_ih, w_hh, b_ih, b_hh, conv_w, fc_w, fc_b, np_mdt):
    """Host-side layout prep (shared across cores + per-core x shards).

    Merged contraction rows (640 = 5 k-tiles of 128):
      tile 0: h[0:128]        tile 1: h[128:256]
      tile 2: h[256:300] at parts 0..43, bias (const-1 row) at part 44,
              x[256:300] at parts 64..107, zeros elsewhere
      tile 3: x[0:128]        tile 4: x[128:256]
    """
    bias = (b_ih + b_hh).astype(np.float32)  # [2, 1200]
    wc = np.zeros((2, NK, 128, 1200), np.float32)
    for d in range(2):
        comb = np.zeros((NK * 128, 1200), np.float32)
        comb[0:256] = w_hh[d].T[0:256]
        comb[256:300] = w_hh[d].T[256:300]
        comb[256 + BIASROW] = bias[d]
        comb[256 + XTAIL:256 + XTAIL + 44] = w_ih[d].T[256:300]
        comb[384:512] = w_ih[d].T[0:128]
        comb[512:640] = w_ih[d].T[128:256]
        wc[d] = comb.reshape(NK, 128, 1200)

    def h_pack(vec_or_mat, width):
        """Pack [300(, width)] h-feature data into the 3-tile h k-layout."""
        out = np.zeros((3, 128, width), np.float32)
        v = vec_or_mat.reshape(H, width)
        out[0] = v[0:128]
        out[1] = v[128:256]
        out[2, 0:44] = v[256:300]
        return out

    convp = np.ascontiguousarray(
        h_pack(conv_w, 1).reshape(3, 128).T)          # [128, 3]
    fcw = np.ascontiguousarray(
        h_pack(fc_w.T, NCLS).transpose(1, 0, 2).reshape(128, 3 * NCLS))

    shared = {
        "wc": wc.astype(np_mdt),
        "convp": convp.astype(np_mdt),
        "fcw": fcw.astype(np_mdt),
        "fcb": fc_b.reshape(1, NCLS).astype(np_mdt),
        "onesrow": np.ones((1, BS), np.float32).astype(np_mdt),
    }
    if USE_DR:
        import ml_dtypes
        np_f8 = ml_dtypes.float8_e4m3
        # wdr[d, p, s, 304*g + r] = w_ih[d][300*g + r, 128*s + p]
        wdr = np.zeros((2, 128, 2, WDRW), np.float32)
        for d in range(2):
            tmp = w_ih[d][:, 0:256].reshape(1200, 2, 128)
            for g in range(4):
                wdr[d, :, :, 304 * g:304 * g + 300] = (
                    tmp[300 * g:300 * g + 300].transpose(2, 1, 0))
        shared["wdr"] = wdr.astype(np_f8)

    # x: [B, H, T] -> per-core [T, 3, 128, BS]:
    # slot 0 = x[0:128], slot 1 = x[128:256],
    # slot 2 = zeros with x[256:300] at parts 64..107.
    xs = np.ascontiguousarray(np.transpose(x, (2, 1, 0)))  # [T, H, B]
    xp = np.zeros((T, 3, 128, B), np.float32)
    xp[:, 0] = xs[:, 0:128]
    xp[:, 1] = xs[:, 128:256]
    xp[:, 2, XTAIL:XTAIL + 44] = xs[:, 256:300]
    xp = xp.reshape(T, 3, 128, NCORES, BS)
    if USE_DR:
        # xdr[t, p, s, b] = x[t, 128*s + p, b]
        xdr = xs[:, 0:256].reshape(T, 2, 128, NCORES, BS).transpose(0, 2, 1, 3, 4)
    in_maps = []
    for c in range(NCORES):
        m = dict(shared)
        m["xt"] = np.ascontiguousarray(xp[:, :, :, c]).astype(np_mdt)
        if USE_DR:
            import ml_dtypes
            m["xdr"] = np.ascontiguousarray(xdr[:, :, :, c]).astype(
                ml_dtypes.float8_e4m3)
        in_maps.append(m)
    return in_maps


def _np_mdt(mdt_name):
    return np.float16 if mdt_name == "float16" else (
        __import__("ml_dtypes").bfloat16 if mdt_name == "bfloat16" else np.float32)


def _runner(repeat=0, variant="full"):
    key = (MM_DT_NAME, repeat, variant)
    if key not in _CACHE:
        _CACHE[key] = _Runner(_build(MM_DT_NAME, repeat=repeat,
                                     variant=variant), NCORES)
    return _CACHE[key]


def _in_maps(inputs_f32):
    return _prep(*inputs_f32, _np_mdt(MM_DT_NAME))


def _inputs_f32(x, w_ih, w_hh, b_ih, b_hh, conv_w, fc_w, fc_b):
    return [np.asarray(a, np.float32) for a in
            (x, w_ih, w_hh, b_ih, b_hh, conv_w, fc_w, fc_b)]


def kernel(x, w_ih, w_hh, b_ih, b_hh, conv_w, fc_w, fc_b):
    runner = _runner(repeat=0)
    in_maps = _in_maps(_inputs_f32(x, w_ih, w_hh, b_ih, b_hh,
                                   conv_w, fc_w, fc_b))
    results = runner.run(in_maps)
    out = np.concatenate([r["out"] for r in results], axis=0)
    return out.astype(np.float32)


def bench(x, w_ih, w_hh, b_ih, b_hh, conv_w, fc_w, fc_b, iters=5):
    runner = _runner(repeat=0)
    in_maps = _in_maps(_inputs_f32(x, w_ih, w_hh, b_ih, b_hh,
                                   conv_w, fc_w, fc_b))
    return runner.bench(in_maps, iters=iters)


def measure_exec_ns(inputs, r_lo=1, r_hi=301, iters=10):
    """Device execution time of one full forward pass, in ns.

    The axon tunnel adds a fixed ~70-80 ms completion-notification latency
    to every blocking call, independent of what the NEFF does (measured:
    a trivial 4-instruction kernel takes the same wall time as the full
    LSTM).  To measure hardware execution, both builds wrap the whole
    T-step forward in a hardware For_i loop (r_lo vs r_hi iterations,
    identical instruction stream per iteration); the slope
    (min_wall(r_hi) - min_wall(r_lo)) / (r_hi - r_lo) is the steady-state
    on-device time of one forward pass with the constant latency cancelled.
    Samples are interleaved so network drift affects both arms equally.
    """
    import time
    in_maps = _in_maps(_inputs_f32(**inputs) if isinstance(inputs, dict)
                       else _inputs_f32(*inputs))
    runners = {rep: _runner(repeat=rep) for rep in (r_lo, r_hi)}
    dev_in = {rep: runners[rep].put_inputs(in_maps) for rep in (r_lo, r_hi)}
    for rep in (r_lo, r_hi):
        runners[rep].call(dev_in[rep])  # warm
    walls = {r_lo: [], r_hi: []}
    for _ in range(iters):
        for rep in (r_lo, r_hi):
            t0 = time.perf_counter()
            runners[rep].call(dev_in[rep])
            walls[rep].append(time.perf_counter() - t0)
    lo, hi = min(walls[r_lo]), min(walls[r_hi])
    ns = (hi - lo) * 1e9 / (r_hi - r_lo)
    return max(int(ns), 1), walls
